# revision 1
# baseline (speedup 1.0000x reference)
"""Trainium2 Bass kernel for AngularMinPooling.

out[v, r] = inputs[v, r, argmin_j ||inputs[v, j, :]||_2]
Input (500000, 8, 64) f32 -> Output (500000, 8) f32.
Vertices are sharded across 8 NeuronCores; no cross-core communication.

Per 128x8-vertex tile: ACT squares the features into a scratch tile
(and copies the first R feature columns to a small fp16 gather tile so
the big input buffer frees early), DVE does the segmented f32 sum-reduce
to squared norms, a min-reduce, an is_le one-hot at the min, and a
one-hot weighted sum for the gather. The one-hot mult + sum run in fp16
(DVE 2x mode): the sum adds exactly one nonzero value to zeros, so the
only precision cost is the final fp16 rounding of the output (~3e-4
rel), while the argmin itself is computed entirely in f32. GpSimd is
deliberately unused: it shares DVE's SBUF port pair, so work moved
there just blocks DVE. Output is staged in fp16 SBUF and written out in
chunks, partition-major (the host undoes the permutation and casts
back).
"""

import os
import sys

import numpy as np

for _p in ("/opt/trn_rl_repo",):
    if os.path.isdir(_p) and _p not in sys.path:
        sys.path.insert(0, _p)

import concourse.bacc as bacc
import concourse.bass as bass
import concourse.tile as tile
from concourse import mybir
from concourse.bass_utils import run_bass_kernel_spmd


def _ensure_ntff_hook():
    """Install the axon NTFF profile hook if the image's antenv lacks it.

    Mirrors trn_boot.py section 6; makes run(..., trace=True) return
    exec_time_ns + perfetto trace instead of silently skipping.
    """
    import types

    try:
        from antenv.axon_hooks import get_axon_ntff_profile_hook  # noqa: F401

        return
    except ImportError:
        pass
    try:
        import antenv
        from trn_agent_boot.trn_boot import _ntff_profile_via_ctypes

        mod = types.ModuleType("antenv.axon_hooks")
        _state = {"hook": None}
        mod.set_axon_ntff_profile_hook = lambda h: _state.__setitem__("hook", h)
        mod.get_axon_ntff_profile_hook = lambda: _state["hook"]
        sys.modules["antenv.axon_hooks"] = mod
        antenv.axon_hooks = mod
        so_path = "/opt/axon/libaxon_pjrt.so"
        if os.path.exists(so_path):
            mod.set_axon_ntff_profile_hook(_ntff_profile_via_ctypes(so_path))
    except Exception:
        pass


_ensure_ntff_hook()

N_VERTICES = 500_000
R = 8
F = 64
N_CORES = 8
N_SHARD = N_VERTICES // N_CORES  # 62500 vertices per core
P = 128  # SBUF partitions
VPP = 8  # vertices per partition per full tile
TILE_V = P * VPP  # 1024 vertices per tile
N_FULL = N_SHARD // TILE_V  # 61 full tiles
TAIL = N_SHARD - N_FULL * TILE_V  # 36 leftover vertices
N_SLOTS = N_FULL * VPP  # 488 staged vertex slots per partition
OUT_CHUNKS = 6  # output DMA chunks (overlap the staged write with compute)

# Warm-up schedule: the first blocks are small sub-tiles so DVE work
# starts as soon as a quarter tile has landed instead of idling ~10us,
# and the 3 tiles' worth of runway builds the DMA-ahead cushion that
# keeps the near-critically balanced pipeline (DVE ~5.4us/tile vs DMA
# ~5.1us/tile) out of its stall limit-cycle.
WARMUP = [2, 2, 4]
# Cool-down: the last tiles are split small so the final tile's serial
# DMA->square->reduce->gather chain (the drain) is ~4us instead of ~14us.
COOLDOWN = [4, 4, 2, 2, 2, 2]
# Full tiles after warm-up are processed in pairs that share one
# min/is_le/mult/sum gather pass, amortizing DVE instruction inits.
# 2 is the sweet spot: bigger groups make DVE consume squares in bursts
# longer than the sqd pool can buffer, stalling ACT once per group
# (good-mode: GROUP=4 -> 414us, 3 -> 378, 2 -> 364, 1 -> 370-372).
GROUP = 2


def _block_schedule():
    """[(v0, n_tiles_or_subtile, slot0, width), ...] covering all slots.

    Warm-up entries have one sub-tile of width<VPP slots; main entries
    are groups of up to GROUP full tiles (width = n*VPP slots).
    """
    blocks = []
    slot = 0
    for vpp in WARMUP:
        blocks.append((slot * P, slot, vpp))
        slot += vpp
    cool = sum(COOLDOWN)
    while slot < N_SLOTS - cool:
        width = min(GROUP * VPP, N_SLOTS - cool - slot)
        blocks.append((slot * P, slot, width))
        slot += width
    for vpp in COOLDOWN:
        blocks.append((slot * P, slot, vpp))
        slot += vpp
    assert slot == N_SLOTS
    return blocks


BLOCKS = _block_schedule()

_DT = mybir.dt.float32
_HT = mybir.dt.float16
_AX = mybir.AxisListType
_OP = mybir.AluOpType


def _build_nc():
    nc = bacc.Bacc(
        "TRN2",
        target_bir_lowering=False,
        debug=False,
        enable_asserts=False,
        num_devices=N_CORES,
    )
    x = nc.dram_tensor("inputs", [N_SHARD, R, F], _DT, kind="ExternalInput")
    # Partition-major staged output: raw[p, t*VPP+v, r] = out[t*TILE_V +
    # p*VPP + v, r]; the host undoes the permutation.
    raw = nc.dram_tensor("raw", [P, N_SLOTS, R], _HT, kind="ExternalOutput")
    traw = nc.dram_tensor("traw", [TAIL, R], _HT, kind="ExternalOutput")
    xa = x.ap()

    with tile.TileContext(nc) as tc:
        with (
            tc.tile_pool(name="xin", bufs=5) as xin_pool,
            tc.tile_pool(name="sqd", bufs=4) as sqd_pool,
            tc.tile_pool(name="x8", bufs=5) as x8_pool,
            tc.tile_pool(name="work", bufs=4) as work_pool,
            tc.tile_pool(name="stage", bufs=1) as stage_pool,
        ):
            stage = stage_pool.tile([P, N_SLOTS, R], _HT)

            GW = GROUP * VPP  # group width in slots

            def head(idx, v0, pc, vpp, sq_dst, x8_dst):
                """Per-tile: DMA in, square, fp16 gather-column copy, big
                f32 F-reduce into the group's norm tile."""
                xt = xin_pool.tile([P, VPP, R, F], _DT, tag="xt")
                src = xa[v0 : v0 + pc * vpp].rearrange("(p v) r f -> p v r f", p=pc)
                dma_eng = nc.sync if idx % 2 == 0 else nc.scalar
                dma_eng.dma_start(out=xt[:pc, :vpp], in_=src)

                sqd = sqd_pool.tile([P, VPP, R, F], _DT, tag="sqd")
                nc.scalar.square(sqd[:pc, :vpp], xt[:pc, :vpp])
                # Early fp16 copy of the R gather columns: frees the big xt
                # buffer after two stages and enables DVE 2x on the gather.
                nc.scalar.copy(x8_dst, xt[:pc, :vpp, :, 0:R])
                nc.vector.tensor_reduce(
                    out=sq_dst, in_=sqd[:pc, :vpp], axis=_AX.X, op=_OP.add
                )

            def gather(pc, w, sqg, x8g, ot_dst):
                """Per-group min / one-hot / gather over w vertex slots."""
                m = work_pool.tile([P, GW], _DT, tag="m")
                nc.vector.tensor_reduce(
                    out=m[:pc, :w], in_=sqg, axis=_AX.X, op=_OP.min
                )
                # One-hot at the min norm (multi-hot only on bitwise-equal
                # ties, which have ~0 probability for random f32 sums).
                sel = work_pool.tile([P, GW, R], _HT, tag="sel")
                nc.vector.tensor_tensor(
                    out=sel[:pc, :w],
                    in0=sqg,
                    in1=m[:pc, :w, None].broadcast_to([pc, w, R]),
                    op=_OP.is_le,
                )
                # Gather via one-hot weighted sum over the first R feature
                # columns (argmin index is always < R). All-fp16 so DVE runs
                # in 2x mode; the sum adds one nonzero to zeros, so fp16
                # costs only the output rounding.
                g = work_pool.tile([P, GW, R, R], _HT, tag="g")
                nc.vector.tensor_tensor(
                    out=g[:pc, :w],
                    in0=x8g,
                    in1=sel[:pc, :w, None, :].broadcast_to([pc, w, R, R]),
                    op=_OP.mult,
                )
                with nc.allow_low_precision(
                    "one-hot sum: adds a single nonzero to zeros, exact"
                ):
                    nc.vector.tensor_reduce(
                        out=ot_dst, in_=g[:pc, :w], axis=_AX.X, op=_OP.add
                    )

            def do_block(idx, v0, width, pc, ot_dst, tail_vpp=None):
                sqg = work_pool.tile([P, GW, R], _DT, tag="sq")
                x8g = x8_pool.tile([P, GW, R, R], _HT, tag="x8")
                if tail_vpp is not None or width < VPP:
                    vpp = tail_vpp if tail_vpp is not None else width
                    w = vpp
                    head(idx, v0, pc, vpp, sqg[:pc, :vpp], x8g[:pc, :vpp])
                else:
                    w = width
                    for k in range(width // VPP):
                        head(
                            idx + k,
                            v0 + k * TILE_V,
                            pc,
                            VPP,
                            sqg[:pc, k * VPP : (k + 1) * VPP],
                            x8g[:pc, k * VPP : (k + 1) * VPP],
                        )
                gather(pc, w, sqg[:pc, :w], x8g[:pc, :w], ot_dst)

            # Chunk boundaries (in slots) for the staged-output DMA. Each
            # chunk is issued one block after its region completes so its
            # semaphore wait is already satisfied and never head-of-line
            # blocks the input-DMA triggers behind it on the sync queue.
            # The last chunks are deliberately small to shorten the drain.
            cool = sum(COOLDOWN)
            fracs = [0.2, 0.4, 0.6, 0.8, 0.94]
            bounds = sorted(
                {round(N_SLOTS * fr / VPP) * VPP for fr in fracs}
                | {N_SLOTS - cool, N_SLOTS}
            )
            chunks = list(zip([0] + bounds[:-1], bounds))
            n_blocks = len(BLOCKS)
            issue_at = {}
            for a, b in chunks:
                done_i = next(
                    i
                    for i, (_, s0, wd) in enumerate(BLOCKS)
                    if s0 + wd >= b
                )
                issue_at.setdefault(min(done_i + 1, n_blocks - 1), []).append(
                    (a, b)
                )
            dma_idx = 0
            for i, (v0, slot0, width) in enumerate(BLOCKS):
                do_block(dma_idx, v0, width, P, stage[:, slot0 : slot0 + width])
                dma_idx += max(1, width // VPP)
                # Tail block runs just before the cool-down so its serial
                # chain hides under the cool-down stream instead of
                # extending the drain.
                if TAIL and i == n_blocks - len(COOLDOWN) - 1:
                    ot_tail = work_pool.tile([P, VPP, R], _HT, tag="ot_tail")
                    do_block(
                        dma_idx, N_FULL * TILE_V, VPP, TAIL,
                        ot_tail[:TAIL, :1], tail_vpp=1,
                    )
                    dma_idx += 1
                for a, b in issue_at.get(i, []):
                    nc.sync.dma_start(
                        out=raw.ap()[:, a:b], in_=stage[:, a:b]
                    )

            if TAIL:
                nc.sync.dma_start(out=traw.ap(), in_=ot_tail[:TAIL, :1])
    nc.finalize()
    return nc


_NC_CACHE = None


def _get_nc():
    global _NC_CACHE
    if _NC_CACHE is None:
        _NC_CACHE = _build_nc()
    return _NC_CACHE


def _decode_raw(raw_arr: np.ndarray, traw_arr: np.ndarray) -> np.ndarray:
    """Map staged [P, N_SLOTS, R] fp16 output back to f32 vertex order."""
    raw_f = np.asarray(raw_arr).astype(np.float32)
    parts = []
    for _v0, slot0, width in BLOCKS:
        if width < VPP:
            parts.append(raw_f[:, slot0 : slot0 + width].reshape(P * width, R))
        else:
            for k in range(width // VPP):
                s = slot0 + k * VPP
                parts.append(raw_f[:, s : s + VPP].reshape(P * VPP, R))
    parts.append(np.asarray(traw_arr).astype(np.float32))
    return np.concatenate(parts, axis=0)


def run(inputs: np.ndarray, **spmd_kwargs):
    inputs = np.ascontiguousarray(np.asarray(inputs, dtype=np.float32))
    assert inputs.shape == (N_VERTICES, R, F), inputs.shape
    shards = np.split(inputs, N_CORES, axis=0)
    in_maps = [{"inputs": np.ascontiguousarray(s)} for s in shards]
    res = run_bass_kernel_spmd(
        _get_nc(), in_maps, core_ids=list(range(N_CORES)), **spmd_kwargs
    )
    out = np.concatenate(
        [_decode_raw(r["raw"], r["traw"]) for r in res.results], axis=0
    )
    return out, res


def kernel(inputs: np.ndarray) -> np.ndarray:
    out, _ = run(inputs)
    return out

